# revision 64
# baseline (speedup 1.0000x reference)
"""GCNConv on 8 TRN2 NeuronCores.

out = rownorm(A + I) @ (x @ W) + b   with A = dense scatter (set semantics)
    = [per dst row r: (sum_{c in dedup(nbr(r))} x[c] + x[r]) / (deg(r)+1)] @ W + b

Hybrid strategy (1D node partition):
  - host: dedup edges, partition dst rows into 8 contiguous blocks of 2048,
    degree-sort rows inside each core block into 16 tiles of 128 rows.
  - The DENSE_T highest-degree tiles are computed by the PE from a
    host-built fp8 block-dense A^T stream (exact 0/1 weights, fp16 x rhs,
    f32 PSUM accumulation over the 128 source blocks) — no per-edge DMA
    descriptors at all.
  - The remaining low-degree tiles use the indirect-DMA gather path:
    padded-CSR per-slot gather of fp16 x rows (one SWDGE descriptor per
    slot), DVE halving-tree segment sum.
  - Shared tail per tile: PE transpose -> S^T, PE matmul S@W, scalar scale
    by 1/(deg+1), DVE bias add, DMA out.
  - host: inverse-permute the 8x2048 row blocks into the full output.
"""

import numpy as np
from contextlib import ExitStack

N = 16384
E = 524288
D = 32
P = 128
NCORES = 8
RPC = N // NCORES          # rows per core = 2048
NTILES = RPC // P          # 16 tiles of 128 rows per core
NBLK = N // P              # 128 source blocks
DENSE_T = 8                # leading (highest-degree) tiles on the dense path
NROUNDS = 4                # gather chunk-emission rounds (chunks per queue)
ZROW = N                   # index of the zeroed pad row in the fp16 scratch
NQ = 4                     # SWDGE queues

_CACHE = {}
_PREP_CACHE = {}


def _plan(Ks, g_tiles):
    """Assign gathered tiles to the NQ SWDGE queues (LPT) and split each
    queue's tiles into exactly NROUNDS chunks, so emissions can go strictly
    round-robin (Pool-DMA emission position % NQ == queue). Returns
    (qlists, chunks) with chunks[t] = [(a, b), ...] slot ranges."""
    nt = len(g_tiles)
    caps = [nt // NQ + (1 if q < nt % NQ else 0) for q in range(NQ)]
    qlists = [[] for _ in range(NQ)]
    for t in sorted(g_tiles, key=lambda t: -Ks[t]):
        cands = [q for q in range(NQ) if len(qlists[q]) < caps[q]]
        q = min(cands, key=lambda i: sum(Ks[x] for x in qlists[i]))
        qlists[q].append(t)
    chunks = {}
    for q in range(NQ):
        m = len(qlists[q])
        for i, t in enumerate(qlists[q]):
            c = NROUNDS // m + (1 if i < NROUNDS % m else 0)
            K = Ks[t]
            if c == 2:
                # skew the split: the last round's desc-gen is the critical
                # path into the endgame, so keep its chunk small
                cuts = [0, (2 * K) // 3, K]
            else:
                cuts = [K * j // c for j in range(c + 1)]
            chunks[t] = [(cuts[j], cuts[j + 1]) for j in range(c)]
    return qlists, chunks
LAST_RESULTS = None        # BassKernelResults of the last run (for test.py)
_TRACE = False             # test.py can flip this for a profiled run

PITCH = 128  # fp16 elems per scratch row = 256B (ISA stride granularity)
FP8_ONE = 0x38  # e4m3 bit pattern for 1.0


def _preprocess(edge_index):
    """Dedup edges; build the dense fp8 A^T stream for the DENSE_T leading
    tiles and a degree-sorted padded-CSR gather schedule for the rest."""
    ei = np.asarray(edge_index)
    key = ei.tobytes()
    if key in _PREP_CACHE:
        return _PREP_CACHE[key]

    dst = ei[0].astype(np.int64)
    src = ei[1].astype(np.int64)
    keys = np.unique(dst * N + src)          # set semantics
    d = (keys // N).astype(np.int64)
    s = (keys % N).astype(np.int32)
    rowptr = np.searchsorted(d, np.arange(N + 1)).astype(np.int64)
    deg = np.diff(rowptr)                    # distinct out-neighbors per row
    slots = (deg + 1).astype(np.int64)       # + self loop
    inv = (1.0 / slots).astype(np.float32)

    # per-core degree-descending row order
    perms = []
    for c in range(NCORES):
        rows = np.arange(c * RPC, (c + 1) * RPC)
        order = np.argsort(-slots[rows], kind="stable")
        perms.append(rows[order])

    g_tiles = list(range(DENSE_T, NTILES))

    # shared (SPMD) per-tile pad width for gathered tiles: max across cores
    Ks = {}
    for t in g_tiles:
        m = max(int(slots[perms[c][t * P]]) for c in range(NCORES))
        Ks[t] = max(m, NROUNDS)
    _, chunk_plan = _plan(Ks, g_tiles)
    offs = {}
    o = 0
    for t in g_tiles:
        offs[t] = o
        o += Ks[t]
    SUMK = o

    idx_arrs, inv_arrs, a8_arrs = [], [], []
    DCOLS = DENSE_T * P
    for c in range(NCORES):
        # ---- gather-path padded CSR ----
        plain = np.full((P, SUMK), ZROW, np.int16)
        invt = np.zeros((P, NTILES), np.float32)
        pc = perms[c]
        for t in range(NTILES):
            for p in range(P):
                invt[p, t] = inv[pc[t * P + p]]
        for t in g_tiles:
            o = offs[t]
            for p in range(P):
                r = int(pc[t * P + p])
                a, b = rowptr[r], rowptr[r + 1]
                k = int(b - a)
                plain[p, o:o + k] = s[a:b]
                plain[p, o + k] = r          # self loop slot
        # dma_gather index format: per gather chunk, gathered position
        # i = j*128 + p reads wrapped[i%16, i//16]; replicated to all 128
        # partitions (one copy per GPSIMD core's partition group).
        idxw = np.empty((16, 8 * SUMK), np.int16)
        for t in g_tiles:
            for (a, b) in chunk_plan[t]:
                o = offs[t] + a
                block = plain[:, o:o + (b - a)]       # [128, Kc]
                flat = block.T.reshape(-1)            # flat[j*128+p] = block[p, j]
                idxw[:, 8 * o:8 * (o + (b - a))] = flat.reshape(-1, 16).T
        idx_arrs.append(np.ascontiguousarray(np.tile(idxw, (8, 1))))
        inv_arrs.append(invt)

        # ---- dense-path fp8 A^T stream, tile-major, partition-contiguous:
        # a8[s, (t*NBLK + b)*P + j] = weight of edge (b*128+s) -> tile t col j.
        # The reference adds eye on top of the scattered adjacency, so rows
        # with an explicit self-edge get diagonal weight 2.
        dense_rows = pc[:DCOLS].astype(np.int64)      # dst col j holds row
        degs = (rowptr[dense_rows + 1] - rowptr[dense_rows]).astype(np.int64)
        srcs = np.concatenate(
            [np.concatenate([s[rowptr[r]:rowptr[r + 1]] for r in dense_rows]),
             dense_rows]                              # self loops
        )
        jcol = np.arange(DCOLS)
        cols = np.concatenate([np.repeat(jcol, degs), jcol])
        t_of = np.concatenate([np.repeat(jcol // P, degs), jcol // P])
        j_of = np.concatenate([np.repeat(jcol % P, degs), jcol % P])
        cnt = np.zeros((P, DENSE_T * NBLK * P), np.uint8)
        np.add.at(cnt, (srcs % P, (t_of * NBLK + srcs // P) * P + j_of), 1)
        a8 = np.where(cnt == 2, 0x40, np.where(cnt == 1, FP8_ONE, 0)).astype(
            np.uint8
        )
        a8_arrs.append(a8)

    prep = {
        "Ks": tuple(Ks[t] for t in g_tiles),
        "SUMK": SUMK,
        "idx": idx_arrs,
        "inv": inv_arrs,
        "a8": a8_arrs,
        "perm": perms,
    }
    _PREP_CACHE[key] = prep
    return prep


_REG_CACHE = {}


def _emit_dma_gather(nc, out_ap, in_ap, idxs_ap, num_idxs, elem_size, elem_step,
                     queue_num=0):
    """bass.dma_gather minus its elem_size_bytes%256 assert (that restriction
    is transpose-only; the real ISA constraint is the source stride, which is
    encoded in 256B units and satisfied by the 256B-pitch scratch)."""
    from concourse import mybir
    from concourse._compat import exact_div

    eng = nc.gpsimd
    assert in_ap.ap[0][0] == elem_step
    stride_bytes = elem_step * mybir.dt.size(in_ap.dtype)
    stride_bytes_256 = exact_div(stride_bytes, 256)
    _in_ap = eng.lower_ap_dma(in_ap, for_custom_bir_dma=True)
    _idxs_ap = eng.lower_ap(idxs_ap)
    _out_ap = eng.lower_ap(out_ap)
    # one num_idxs register per distinct value: each fresh register costs a
    # ~400ns Pool-sequencer MOVE, all hoisted ahead of the first gather
    rk = (id(nc), num_idxs)
    if rk not in _REG_CACHE:
        _REG_CACHE[rk] = eng.to_reg(num_idxs)
    return eng.add_instruction(
        mybir.InstDMAGatherAnt(
            name=nc.get_next_instruction_name(),
            ins=[*_in_ap, _idxs_ap, eng.lower_val_access(_REG_CACHE[rk])],
            outs=[_out_ap],
            transpose=False,
            num_idxs=num_idxs,
            elem_size=elem_size,
            stride_bytes_256=stride_bytes_256,
            gen_mode=0,
            single_packet=False,
            queue_num=queue_num,
            sbuf_tokens_per_rank=0,
            sbuf_free_dim_per_rank=0,
            sbuf_free_dim_pad_per_rank=0,
            sbuf_byte_offset=0,
        )
    )


def _build(Ks, SUMK):
    """Build + compile the (identical-across-cores) Bass program."""
    from concourse import bass, bacc, mybir, tile

    ck = (Ks, SUMK)
    if ck in _CACHE:
        return _CACHE[ck]

    f32 = mybir.dt.float32
    f16 = mybir.dt.float16
    f8 = mybir.dt.float8e4
    i16 = mybir.dt.int16

    g_tiles = list(range(DENSE_T, NTILES))
    Kof = {t: Ks[i] for i, t in enumerate(g_tiles)}
    offs = {}
    o = 0
    for t in g_tiles:
        offs[t] = o
        o += Kof[t]
    DCOLS = DENSE_T * P

    nc = bacc.Bacc(
        "TRN2",
        target_bir_lowering=False,
        debug=False,
        enable_asserts=False,
        num_devices=NCORES,
        num_swdge_queues=NQ,
        dynamic_dma_scratch_size=65536,
    )

    x16_d = nc.dram_tensor("x16s", [N + 1, PITCH], f16, kind="ExternalInput").ap()
    idx_d = nc.dram_tensor("idx", [P, 8 * SUMK], i16, kind="ExternalInput").ap()
    inv_d = nc.dram_tensor("inv", [P, NTILES], f32, kind="ExternalInput").ap()
    # stacked weight/bias: rows 0:2D+1 = [W; W; b], rows 2D+1: = [W; b]
    w_d = nc.dram_tensor("wstk", [3 * D + 2, D], f32, kind="ExternalInput").ap()
    ident_d = nc.dram_tensor("ident", [P, P], f32, kind="ExternalInput").ap()
    a8_d = nc.dram_tensor(
        "a8", [P, DENSE_T * NBLK * P], f8, kind="ExternalInput"
    ).ap()
    # fp8 hi/lo split of x per source block: [x_hi_b | x_lo_b] pairs of D cols
    xblk_d = nc.dram_tensor("xblk", [P, NBLK * 2 * D], f8, kind="ExternalInput").ap()
    out_d = nc.dram_tensor("out", [RPC, D], f32, kind="ExternalOutput").ap()

    with tile.TileContext(nc) as tc, ExitStack() as ctx:
        const = ctx.enter_context(tc.tile_pool(name="const", bufs=1))
        gp = ctx.enter_context(tc.tile_pool(name="gp", bufs=6))
        ap_ = ctx.enter_context(tc.tile_pool(name="ap", bufs=3))
        sp = ctx.enter_context(tc.tile_pool(name="sp", bufs=3))
        tp = ctx.enter_context(tc.tile_pool(name="tp", bufs=3))
        op_ = ctx.enter_context(tc.tile_pool(name="op", bufs=3))
        ppt = ctx.enter_context(tc.tile_pool(name="ppt", bufs=2, space="PSUM"))
        ppm = ctx.enter_context(tc.tile_pool(name="ppm", bufs=2, space="PSUM"))
        ppd = ctx.enter_context(tc.tile_pool(name="ppd", bufs=1, space="PSUM"))

        # constants (idx first: the gather leg is the long pole and waits on it)
        idx_sb = const.tile([P, 8 * SUMK], i16)
        nc.sync.dma_start(out=idx_sb[:], in_=idx_d[:])
        xblk_sb = const.tile([P, NBLK * 2 * D], f8)
        nc.scalar.dma_start(out=xblk_sb[:], in_=xblk_d[:])
        wbd_sb = const.tile([2 * D + 1, D], f32)
        nc.sync.dma_start(out=wbd_sb[:], in_=w_d[0:2 * D + 1, :])
        wbg_sb = const.tile([D + 1, D], f32)
        nc.sync.dma_start(out=wbg_sb[:], in_=w_d[2 * D + 1:3 * D + 2, :])
        ones1 = const.tile([1, P], f32)
        nc.vector.memset(ones1[:], 1.0)
        inv_sb = const.tile([P, NTILES], f32)
        nc.sync.dma_start(out=inv_sb[:], in_=inv_d[:])
        # identity via DMA (a host input): building it with gpsimd iota/memset
        # forces a second ~6us ext-isa IRAM library load onto the Pool engine
        # ahead of the first gather
        ident = const.tile([P, P], f32)
        nc.sync.dma_start(out=ident[:], in_=ident_d[:])

        def tail(t, psrc_ap, w):
            # Sd = rowscale(psrc, 1/(deg+1)); out = [Sd | 1] @ [W(2); b].
            # Scalar + PE only (no DVE): bias rides the W matmul via an
            # appended ones row, inv scaling rides the PSUM->SBUF copy.
            Sd = sp.tile([P, w], f32, tag="S")
            nc.scalar.activation(
                out=Sd[:],
                in_=psrc_ap,
                func=mybir.ActivationFunctionType.Copy,
                scale=inv_sb[:, t:t + 1],
            )
            pT = ppt.tile([w, P], f32, tag="pT")
            nc.tensor.transpose(out=pT[:], in_=Sd[:], identity=ident[:])
            ST = tp.tile([w + 1, P], f32, tag="ST")
            nc.scalar.copy(out=ST[0:w, :], in_=pT[:])
            nc.scalar.copy(out=ST[w:w + 1, :], in_=ones1[:])
            pO = ppm.tile([P, D], f32, tag="pO")
            wrows = wbd_sb[:] if w == 2 * D else wbg_sb[:]
            nc.tensor.matmul(
                out=pO[:], lhsT=ST[:], rhs=wrows, start=True, stop=True
            )
            O = op_.tile([P, D], f32, tag="O")
            nc.scalar.copy(out=O[:], in_=pO[:])
            nc.sync.dma_start(out=out_d[t * P:(t + 1) * P, :], in_=O[:])

        # ---------------- dense path: PSUM[t] = sum_b A^T[b,t] @ x[b] -------
        # Tile-major streaming: one PSUM accumulation group open at a time
        # per bank (start=True clears has_written for the WHOLE bank, so
        # interleaved groups in one bank corrupt each other).
        # 4KB per-partition descriptors: SDMA engines round-robin between
        # queues at packet boundaries, so bigger descs starve the concurrent
        # SWDGE gather drains
        NBC = 32                      # blocks per A^T stream chunk (512KB)
        for t in range(DENSE_T):
            psum_t = ppd.tile([P, 2 * D], f32, tag="pd", bufs=2)
            for i in range(NBLK // NBC):
                a_sb = ap_.tile([P, NBC * P], f8, tag="a8", bufs=4)
                eng = nc.sync if (t * (NBLK // NBC) + i) % 2 else nc.scalar
                eng.dma_start(
                    out=a_sb[:],
                    in_=a8_d[:, (t * NBLK + i * NBC) * P:
                             (t * NBLK + (i + 1) * NBC) * P],
                )
                for k in range(NBC):
                    b = i * NBC + k
                    nc.tensor.matmul(
                        out=psum_t[:],
                        lhsT=a_sb[:, k * P:(k + 1) * P],
                        rhs=xblk_sb[:, b * 2 * D:(b + 1) * 2 * D],
                        start=(b == 0),
                        stop=(b == NBLK - 1),
                    )
            # shared tail; the fp8 hi/lo halves recombine inside the W matmul
            # (lhsT = [S_hi^T; S_lo^T], rhs = [W; W])
            tail(t, psum_t[:], 2 * D)

        # ---------------- gather path for the low-degree tiles --------------
        # Balance the NQ SWDGE queues by descriptor count (LPT), split each
        # queue's tiles into NROUNDS chunks, emit strictly round-robin so the
        # Pool-DMA emission position stays congruent with the queue number.
        qlists, chunk_plan = _plan(Kof, g_tiles)
        qchunks = [
            [(t, rng) for t in qlists[q] for rng in chunk_plan[t]]
            for q in range(NQ)
        ]
        nleft = {t: len(chunk_plan[t]) for t in g_tiles}

        def emit_chunk(t, a, b, q):
            o = offs[t] + a
            Kc = b - a
            G = Gt_of[t]
            _emit_dma_gather(
                nc,
                out_ap=G[:, a * D:b * D].rearrange("p (k d) -> p k d", d=D),
                in_ap=x16_d[:, 0:D],
                idxs_ap=idx_sb[:, 8 * o:8 * (o + Kc)],
                num_idxs=P * Kc,
                elem_size=D,
                elem_step=PITCH,
                queue_num=q,
            )

        Gt_of = {
            t: gp.tile([P, Kof[t] * D], f16, tag="G", name=f"G{t}", bufs=8)
            for t in g_tiles
        }
        def reduce_range(Gt, a, b):
            # halving-tree sum of slot blocks [a, b) into slot a (fp16)
            cur = b - a
            while cur > 1:
                if cur % 2 == 1:
                    nc.vector.tensor_add(
                        out=Gt[:, a * D:(a + 1) * D],
                        in0=Gt[:, a * D:(a + 1) * D],
                        in1=Gt[:, (a + cur - 1) * D:(a + cur) * D],
                    )
                    cur -= 1
                else:
                    m = cur // 2
                    nc.vector.tensor_add(
                        out=Gt[:, a * D:(a + m) * D],
                        in0=Gt[:, a * D:(a + m) * D],
                        in1=Gt[:, (a + m) * D:(a + 2 * m) * D],
                    )
                    cur = m

        done_chunks = {t: [] for t in g_tiles}
        for r in range(NROUNDS):
            for q in range(NQ):
                t, (a, b) = qchunks[q][r]
                emit_chunk(t, a, b, q)
            for q in range(NQ):
                t, (a, b) = qchunks[q][r]
                # reduce each chunk as soon as it drains; the final combine
                # and tail only wait for the last chunk
                reduce_range(Gt_of[t], a, b)
                done_chunks[t].append(a)
                nleft[t] -= 1
                if nleft[t]:
                    continue
                Gt = Gt_of[t]
                heads = done_chunks[t]
                S = sp.tile([P, D], f32, tag="S")
                nc.vector.tensor_add(
                    out=S[:],
                    in0=Gt[:, heads[0] * D:(heads[0] + 1) * D],
                    in1=Gt[:, heads[1] * D:(heads[1] + 1) * D],
                )
                for h in heads[2:]:
                    nc.vector.tensor_add(
                        out=S[:], in0=S[:], in1=Gt[:, h * D:(h + 1) * D]
                    )
                # Push the gather tails to the end of every engine's schedule
                # (the scheduler's cost model underestimates the gather DMAs,
                # and an early-queued tail op blocks the engine behind a long
                # semaphore wait).
                with tc.tile_wait_until(1.0):
                    tail(t, S[:], D)

    nc.compile()
    _CACHE[ck] = nc
    return nc


def kernel(**inputs):
    global LAST_RESULTS
    import ml_dtypes
    from concourse import bass_utils

    x = np.ascontiguousarray(np.asarray(inputs["x"], dtype=np.float32))
    edge_index = np.asarray(inputs["edge_index"])
    weight = np.ascontiguousarray(np.asarray(inputs["weight"], dtype=np.float32))
    bias = np.asarray(inputs["bias"], dtype=np.float32)

    prep = _preprocess(edge_index)
    nc = _build(prep["Ks"], prep["SUMK"])

    x16 = x.astype(np.float16)
    # pre-padded fp16 x at 256B row pitch, with a zeroed pad row at index N
    xpad = np.zeros((N + 1, PITCH), dtype=np.float16)
    xpad[:N, :D] = x16
    # fp8 hi/lo split per source block for the dense rhs:
    # xblk[p, b*2D:(b*2+1)D] = fp8(x[b*128+p]), next D cols = fp8 residual
    f8t = ml_dtypes.float8_e4m3
    xh = x.astype(f8t)
    xl = (x - xh.astype(np.float32)).astype(f8t)
    xblk = np.ascontiguousarray(
        np.concatenate(
            [xh.reshape(NBLK, P, 1, D), xl.reshape(NBLK, P, 1, D)], axis=2
        ).transpose(1, 0, 2, 3).reshape(P, NBLK * 2 * D)
    )

    in_maps = [
        {
            "x16s": xpad,
            "idx": prep["idx"][c],
            "inv": prep["inv"][c],
            "wstk": np.ascontiguousarray(
                np.vstack([weight, weight, bias[None], weight, bias[None]])
            ).astype(np.float32),
            "ident": np.eye(P, dtype=np.float32),
            "a8": prep["a8"][c].view(ml_dtypes.float8_e4m3),
            "xblk": xblk,
        }
        for c in range(NCORES)
    ]

    res = bass_utils.run_bass_kernel_spmd(
        nc, in_maps, core_ids=list(range(NCORES)), trace=_TRACE
    )
    LAST_RESULTS = res

    out = np.empty((N, D), dtype=np.float32)
    for c in range(NCORES):
        out[prep["perm"][c]] = res.results[c]["out"]
    return out


# revision 66
# speedup vs baseline: 1.1390x; 1.1390x over previous
"""GCNConv on 8 TRN2 NeuronCores.

out = rownorm(A + I) @ (x @ W) + b   with A = dense scatter (set semantics)
    = [per dst row r: (sum_{c in dedup(nbr(r))} x[c] + x[r]) / (deg(r)+1)] @ W + b

Hybrid strategy (1D node partition):
  - host: dedup edges, partition dst rows into 8 contiguous blocks of 2048,
    degree-sort rows inside each core block into 16 tiles of 128 rows.
  - The DENSE_T highest-degree tiles are computed by the PE from a
    host-built fp8 block-dense A^T stream (exact 0/1 weights, fp16 x rhs,
    f32 PSUM accumulation over the 128 source blocks) — no per-edge DMA
    descriptors at all.
  - The remaining low-degree tiles use the indirect-DMA gather path:
    padded-CSR per-slot gather of fp16 x rows (one SWDGE descriptor per
    slot), DVE halving-tree segment sum.
  - Shared tail per tile: PE transpose -> S^T, PE matmul S@W, scalar scale
    by 1/(deg+1), DVE bias add, DMA out.
  - host: inverse-permute the 8x2048 row blocks into the full output.
"""

import numpy as np
from contextlib import ExitStack

N = 16384
E = 524288
D = 32
P = 128
NCORES = 8
RPC = N // NCORES          # rows per core = 2048
NTILES = RPC // P          # 16 tiles of 128 rows per core
NBLK = N // P              # 128 source blocks
DENSE_T = 8                # leading (highest-degree) tiles on the dense path
NROUNDS = 4                # gather chunk-emission rounds (chunks per queue)
ZROW = N                   # index of the zeroed pad row in the fp16 scratch
NQ = 4                     # SWDGE queues

_CACHE = {}
_PREP_CACHE = {}


def _plan(Ks, g_tiles):
    """Assign gathered tiles to the NQ SWDGE queues (LPT) and split each
    queue's tiles into exactly NROUNDS chunks, so emissions can go strictly
    round-robin (Pool-DMA emission position % NQ == queue). Returns
    (qlists, chunks) with chunks[t] = [(a, b), ...] slot ranges."""
    nt = len(g_tiles)
    caps = [nt // NQ + (1 if q < nt % NQ else 0) for q in range(NQ)]
    qlists = [[] for _ in range(NQ)]
    for t in sorted(g_tiles, key=lambda t: -Ks[t]):
        cands = [q for q in range(NQ) if len(qlists[q]) < caps[q]]
        q = min(cands, key=lambda i: sum(Ks[x] for x in qlists[i]))
        qlists[q].append(t)
    chunks = {}
    for q in range(NQ):
        m = len(qlists[q])
        for i, t in enumerate(qlists[q]):
            c = NROUNDS // m + (1 if i < NROUNDS % m else 0)
            K = Ks[t]
            if c == 2:
                # skew the split: the last round's desc-gen is the critical
                # path into the endgame, so keep its chunk small
                cuts = [0, (3 * K) // 5, K]
            else:
                cuts = [K * j // c for j in range(c + 1)]
            chunks[t] = [(cuts[j], cuts[j + 1]) for j in range(c)]
    return qlists, chunks
LAST_RESULTS = None        # BassKernelResults of the last run (for test.py)
_TRACE = False             # test.py can flip this for a profiled run

PITCH = 128  # fp16 elems per scratch row = 256B (ISA stride granularity)
FP8_ONE = 0x38  # e4m3 bit pattern for 1.0


def _preprocess(edge_index):
    """Dedup edges; build the dense fp8 A^T stream for the DENSE_T leading
    tiles and a degree-sorted padded-CSR gather schedule for the rest."""
    ei = np.asarray(edge_index)
    key = ei.tobytes()
    if key in _PREP_CACHE:
        return _PREP_CACHE[key]

    dst = ei[0].astype(np.int64)
    src = ei[1].astype(np.int64)
    keys = np.unique(dst * N + src)          # set semantics
    d = (keys // N).astype(np.int64)
    s = (keys % N).astype(np.int32)
    rowptr = np.searchsorted(d, np.arange(N + 1)).astype(np.int64)
    deg = np.diff(rowptr)                    # distinct out-neighbors per row
    slots = (deg + 1).astype(np.int64)       # + self loop
    inv = (1.0 / slots).astype(np.float32)

    # per-core degree-descending row order
    perms = []
    for c in range(NCORES):
        rows = np.arange(c * RPC, (c + 1) * RPC)
        order = np.argsort(-slots[rows], kind="stable")
        perms.append(rows[order])

    g_tiles = list(range(DENSE_T, NTILES))

    # shared (SPMD) per-tile pad width for gathered tiles: max across cores
    Ks = {}
    for t in g_tiles:
        m = max(int(slots[perms[c][t * P]]) for c in range(NCORES))
        Ks[t] = max(m, NROUNDS)
    _, chunk_plan = _plan(Ks, g_tiles)
    offs = {}
    o = 0
    for t in g_tiles:
        offs[t] = o
        o += Ks[t]
    SUMK = o

    idx_arrs, inv_arrs, a8_arrs = [], [], []
    DCOLS = DENSE_T * P
    for c in range(NCORES):
        # ---- gather-path padded CSR ----
        plain = np.full((P, SUMK), ZROW, np.int16)
        invt = np.zeros((P, NTILES), np.float32)
        pc = perms[c]
        for t in range(NTILES):
            for p in range(P):
                invt[p, t] = inv[pc[t * P + p]]
        for t in g_tiles:
            o = offs[t]
            for p in range(P):
                r = int(pc[t * P + p])
                a, b = rowptr[r], rowptr[r + 1]
                k = int(b - a)
                plain[p, o:o + k] = s[a:b]
                plain[p, o + k] = r          # self loop slot
        # dma_gather index format: per gather chunk, gathered position
        # i = j*128 + p reads wrapped[i%16, i//16]; replicated to all 128
        # partitions (one copy per GPSIMD core's partition group).
        idxw = np.empty((16, 8 * SUMK), np.int16)
        for t in g_tiles:
            for (a, b) in chunk_plan[t]:
                o = offs[t] + a
                block = plain[:, o:o + (b - a)]       # [128, Kc]
                flat = block.T.reshape(-1)            # flat[j*128+p] = block[p, j]
                idxw[:, 8 * o:8 * (o + (b - a))] = flat.reshape(-1, 16).T
        idx_arrs.append(np.ascontiguousarray(np.tile(idxw, (8, 1))))
        inv_arrs.append(invt)

        # ---- dense-path fp8 A^T stream, tile-major, partition-contiguous:
        # a8[s, (t*NBLK + b)*P + j] = weight of edge (b*128+s) -> tile t col j.
        # The reference adds eye on top of the scattered adjacency, so rows
        # with an explicit self-edge get diagonal weight 2.
        dense_rows = pc[:DCOLS].astype(np.int64)      # dst col j holds row
        degs = (rowptr[dense_rows + 1] - rowptr[dense_rows]).astype(np.int64)
        srcs = np.concatenate(
            [np.concatenate([s[rowptr[r]:rowptr[r + 1]] for r in dense_rows]),
             dense_rows]                              # self loops
        )
        jcol = np.arange(DCOLS)
        cols = np.concatenate([np.repeat(jcol, degs), jcol])
        t_of = np.concatenate([np.repeat(jcol // P, degs), jcol // P])
        j_of = np.concatenate([np.repeat(jcol % P, degs), jcol % P])
        cnt = np.zeros((P, DENSE_T * NBLK * P), np.uint8)
        np.add.at(cnt, (srcs % P, (t_of * NBLK + srcs // P) * P + j_of), 1)
        a8 = np.where(cnt == 2, 0x40, np.where(cnt == 1, FP8_ONE, 0)).astype(
            np.uint8
        )
        a8_arrs.append(a8)

    prep = {
        "Ks": tuple(Ks[t] for t in g_tiles),
        "SUMK": SUMK,
        "idx": idx_arrs,
        "inv": inv_arrs,
        "a8": a8_arrs,
        "perm": perms,
    }
    _PREP_CACHE[key] = prep
    return prep


_REG_CACHE = {}


def _emit_dma_gather(nc, out_ap, in_ap, idxs_ap, num_idxs, elem_size, elem_step,
                     queue_num=0):
    """bass.dma_gather minus its elem_size_bytes%256 assert (that restriction
    is transpose-only; the real ISA constraint is the source stride, which is
    encoded in 256B units and satisfied by the 256B-pitch scratch)."""
    from concourse import mybir
    from concourse._compat import exact_div

    eng = nc.gpsimd
    assert in_ap.ap[0][0] == elem_step
    stride_bytes = elem_step * mybir.dt.size(in_ap.dtype)
    stride_bytes_256 = exact_div(stride_bytes, 256)
    _in_ap = eng.lower_ap_dma(in_ap, for_custom_bir_dma=True)
    _idxs_ap = eng.lower_ap(idxs_ap)
    _out_ap = eng.lower_ap(out_ap)
    # one num_idxs register per distinct value: each fresh register costs a
    # ~400ns Pool-sequencer MOVE, all hoisted ahead of the first gather
    rk = (id(nc), num_idxs)
    if rk not in _REG_CACHE:
        _REG_CACHE[rk] = eng.to_reg(num_idxs)
    return eng.add_instruction(
        mybir.InstDMAGatherAnt(
            name=nc.get_next_instruction_name(),
            ins=[*_in_ap, _idxs_ap, eng.lower_val_access(_REG_CACHE[rk])],
            outs=[_out_ap],
            transpose=False,
            num_idxs=num_idxs,
            elem_size=elem_size,
            stride_bytes_256=stride_bytes_256,
            gen_mode=0,
            single_packet=False,
            queue_num=queue_num,
            sbuf_tokens_per_rank=0,
            sbuf_free_dim_per_rank=0,
            sbuf_free_dim_pad_per_rank=0,
            sbuf_byte_offset=0,
        )
    )


def _build(Ks, SUMK):
    """Build + compile the (identical-across-cores) Bass program."""
    from concourse import bass, bacc, mybir, tile

    ck = (Ks, SUMK)
    if ck in _CACHE:
        return _CACHE[ck]

    f32 = mybir.dt.float32
    f16 = mybir.dt.float16
    f8 = mybir.dt.float8e4
    i16 = mybir.dt.int16

    g_tiles = list(range(DENSE_T, NTILES))
    Kof = {t: Ks[i] for i, t in enumerate(g_tiles)}
    offs = {}
    o = 0
    for t in g_tiles:
        offs[t] = o
        o += Kof[t]
    DCOLS = DENSE_T * P

    nc = bacc.Bacc(
        "TRN2",
        target_bir_lowering=False,
        debug=False,
        enable_asserts=False,
        num_devices=NCORES,
        num_swdge_queues=NQ,
        dynamic_dma_scratch_size=65536,
    )

    x16_d = nc.dram_tensor("x16s", [N + 1, PITCH], f16, kind="ExternalInput").ap()
    idx_d = nc.dram_tensor("idx", [P, 8 * SUMK], i16, kind="ExternalInput").ap()
    inv_d = nc.dram_tensor("inv", [P, NTILES], f32, kind="ExternalInput").ap()
    # stacked weight/bias: rows 0:2D+1 = [W; W; b], rows 2D+1: = [W; b]
    w_d = nc.dram_tensor("wstk", [3 * D + 2, D], f32, kind="ExternalInput").ap()
    ident_d = nc.dram_tensor("ident", [P, P], f32, kind="ExternalInput").ap()
    a8_d = nc.dram_tensor(
        "a8", [P, DENSE_T * NBLK * P], f8, kind="ExternalInput"
    ).ap()
    # fp8 hi/lo split of x per source block: [x_hi_b | x_lo_b] pairs of D cols
    xblk_d = nc.dram_tensor("xblk", [P, NBLK * 2 * D], f8, kind="ExternalInput").ap()
    out_d = nc.dram_tensor("out", [RPC, D], f32, kind="ExternalOutput").ap()

    with tile.TileContext(nc) as tc, ExitStack() as ctx:
        const = ctx.enter_context(tc.tile_pool(name="const", bufs=1))
        gp = ctx.enter_context(tc.tile_pool(name="gp", bufs=6))
        ap_ = ctx.enter_context(tc.tile_pool(name="ap", bufs=3))
        sp = ctx.enter_context(tc.tile_pool(name="sp", bufs=3))
        tp = ctx.enter_context(tc.tile_pool(name="tp", bufs=3))
        op_ = ctx.enter_context(tc.tile_pool(name="op", bufs=3))
        ppt = ctx.enter_context(tc.tile_pool(name="ppt", bufs=2, space="PSUM"))
        ppm = ctx.enter_context(tc.tile_pool(name="ppm", bufs=2, space="PSUM"))
        ppd = ctx.enter_context(tc.tile_pool(name="ppd", bufs=1, space="PSUM"))

        # constants (idx first: the gather leg is the long pole and waits on it)
        idx_sb = const.tile([P, 8 * SUMK], i16)
        nc.sync.dma_start(out=idx_sb[:], in_=idx_d[:])
        xblk_sb = const.tile([P, NBLK * 2 * D], f8)
        nc.scalar.dma_start(out=xblk_sb[:], in_=xblk_d[:])
        wbd_sb = const.tile([2 * D + 1, D], f32)
        nc.sync.dma_start(out=wbd_sb[:], in_=w_d[0:2 * D + 1, :])
        wbg_sb = const.tile([D + 1, D], f32)
        nc.sync.dma_start(out=wbg_sb[:], in_=w_d[2 * D + 1:3 * D + 2, :])
        ones1 = const.tile([1, P], f32)
        nc.vector.memset(ones1[:], 1.0)
        inv_sb = const.tile([P, NTILES], f32)
        nc.sync.dma_start(out=inv_sb[:], in_=inv_d[:])
        # identity via DMA (a host input): building it with gpsimd iota/memset
        # forces a second ~6us ext-isa IRAM library load onto the Pool engine
        # ahead of the first gather
        ident = const.tile([P, P], f32)
        nc.sync.dma_start(out=ident[:], in_=ident_d[:])

        def tail(t, psrc_ap, w):
            # Sd = rowscale(psrc, 1/(deg+1)); out = [Sd | 1] @ [W(2); b].
            # Scalar + PE only (no DVE): bias rides the W matmul via an
            # appended ones row, inv scaling rides the PSUM->SBUF copy.
            Sd = sp.tile([P, w], f32, tag="S")
            nc.scalar.activation(
                out=Sd[:],
                in_=psrc_ap,
                func=mybir.ActivationFunctionType.Copy,
                scale=inv_sb[:, t:t + 1],
            )
            pT = ppt.tile([w, P], f32, tag="pT")
            nc.tensor.transpose(out=pT[:], in_=Sd[:], identity=ident[:])
            ST = tp.tile([w + 1, P], f32, tag="ST")
            nc.scalar.copy(out=ST[0:w, :], in_=pT[:])
            nc.scalar.copy(out=ST[w:w + 1, :], in_=ones1[:])
            pO = ppm.tile([P, D], f32, tag="pO")
            wrows = wbd_sb[:] if w == 2 * D else wbg_sb[:]
            nc.tensor.matmul(
                out=pO[:], lhsT=ST[:], rhs=wrows, start=True, stop=True
            )
            O = op_.tile([P, D], f32, tag="O")
            nc.scalar.copy(out=O[:], in_=pO[:])
            nc.sync.dma_start(out=out_d[t * P:(t + 1) * P, :], in_=O[:])

        # ---------------- dense path: PSUM[t] = sum_b A^T[b,t] @ x[b] -------
        # Tile-major streaming: one PSUM accumulation group open at a time
        # per bank (start=True clears has_written for the WHOLE bank, so
        # interleaved groups in one bank corrupt each other).
        # 4KB per-partition descriptors: SDMA engines round-robin between
        # queues at packet boundaries, so bigger descs starve the concurrent
        # SWDGE gather drains
        NBC = 32                      # blocks per A^T stream chunk (512KB)
        for t in range(DENSE_T):
            psum_t = ppd.tile([P, 2 * D], f32, tag="pd", bufs=2)
            for i in range(NBLK // NBC):
                a_sb = ap_.tile([P, NBC * P], f8, tag="a8", bufs=4)
                eng = nc.sync if (t * (NBLK // NBC) + i) % 2 else nc.scalar
                eng.dma_start(
                    out=a_sb[:],
                    in_=a8_d[:, (t * NBLK + i * NBC) * P:
                             (t * NBLK + (i + 1) * NBC) * P],
                )
                for k in range(NBC):
                    b = i * NBC + k
                    nc.tensor.matmul(
                        out=psum_t[:],
                        lhsT=a_sb[:, k * P:(k + 1) * P],
                        rhs=xblk_sb[:, b * 2 * D:(b + 1) * 2 * D],
                        start=(b == 0),
                        stop=(b == NBLK - 1),
                    )
            # shared tail; the fp8 hi/lo halves recombine inside the W matmul
            # (lhsT = [S_hi^T; S_lo^T], rhs = [W; W])
            tail(t, psum_t[:], 2 * D)

        # ---------------- gather path for the low-degree tiles --------------
        # Balance the NQ SWDGE queues by descriptor count (LPT), split each
        # queue's tiles into NROUNDS chunks, emit strictly round-robin so the
        # Pool-DMA emission position stays congruent with the queue number.
        qlists, chunk_plan = _plan(Kof, g_tiles)
        qchunks = [
            [(t, rng) for t in qlists[q] for rng in chunk_plan[t]]
            for q in range(NQ)
        ]
        nleft = {t: len(chunk_plan[t]) for t in g_tiles}

        def emit_chunk(t, a, b, q):
            o = offs[t] + a
            Kc = b - a
            G = Gt_of[t]
            _emit_dma_gather(
                nc,
                out_ap=G[:, a * D:b * D].rearrange("p (k d) -> p k d", d=D),
                in_ap=x16_d[:, 0:D],
                idxs_ap=idx_sb[:, 8 * o:8 * (o + Kc)],
                num_idxs=P * Kc,
                elem_size=D,
                elem_step=PITCH,
                queue_num=q,
            )

        Gt_of = {
            t: gp.tile([P, Kof[t] * D], f16, tag="G", name=f"G{t}", bufs=8)
            for t in g_tiles
        }
        def reduce_range(Gt, a, b):
            # halving-tree sum of slot blocks [a, b) into slot a (fp16)
            cur = b - a
            while cur > 1:
                if cur % 2 == 1:
                    nc.vector.tensor_add(
                        out=Gt[:, a * D:(a + 1) * D],
                        in0=Gt[:, a * D:(a + 1) * D],
                        in1=Gt[:, (a + cur - 1) * D:(a + cur) * D],
                    )
                    cur -= 1
                else:
                    m = cur // 2
                    nc.vector.tensor_add(
                        out=Gt[:, a * D:(a + m) * D],
                        in0=Gt[:, a * D:(a + m) * D],
                        in1=Gt[:, (a + m) * D:(a + 2 * m) * D],
                    )
                    cur = m

        done_chunks = {t: [] for t in g_tiles}
        for r in range(NROUNDS):
            for q in range(NQ):
                t, (a, b) = qchunks[q][r]
                emit_chunk(t, a, b, q)
            for q in range(NQ):
                t, (a, b) = qchunks[q][r]
                done_chunks[t].append(a)
                nleft[t] -= 1
                if nleft[t]:
                    # non-final chunk: reduce as soon as it drains (slack)
                    reduce_range(Gt_of[t], a, b)
                    continue
                Gt = Gt_of[t]
                heads = done_chunks[t]
                # Final chunk's reduce, the combine, and the tail all go to
                # the end of every engine's schedule (the cost model
                # underestimates the gather DMAs; an early-queued op blocks
                # the engine behind a long semaphore wait). Within the
                # deferred block, ops sort by emission priority = drain
                # order, so the DVE drains tiles in the order they complete.
                with tc.tile_wait_until(1.0):
                    reduce_range(Gt, a, b)
                    S = sp.tile([P, D], f32, tag="S")
                    nc.vector.tensor_add(
                        out=S[:],
                        in0=Gt[:, heads[0] * D:(heads[0] + 1) * D],
                        in1=Gt[:, heads[1] * D:(heads[1] + 1) * D],
                    )
                    for h in heads[2:]:
                        nc.vector.tensor_add(
                            out=S[:], in0=S[:], in1=Gt[:, h * D:(h + 1) * D]
                        )
                    tail(t, S[:], D)

    nc.compile()
    _CACHE[ck] = nc
    return nc


def kernel(**inputs):
    global LAST_RESULTS
    import ml_dtypes
    from concourse import bass_utils

    x = np.ascontiguousarray(np.asarray(inputs["x"], dtype=np.float32))
    edge_index = np.asarray(inputs["edge_index"])
    weight = np.ascontiguousarray(np.asarray(inputs["weight"], dtype=np.float32))
    bias = np.asarray(inputs["bias"], dtype=np.float32)

    prep = _preprocess(edge_index)
    nc = _build(prep["Ks"], prep["SUMK"])

    x16 = x.astype(np.float16)
    # pre-padded fp16 x at 256B row pitch, with a zeroed pad row at index N
    xpad = np.zeros((N + 1, PITCH), dtype=np.float16)
    xpad[:N, :D] = x16
    # fp8 hi/lo split per source block for the dense rhs:
    # xblk[p, b*2D:(b*2+1)D] = fp8(x[b*128+p]), next D cols = fp8 residual
    f8t = ml_dtypes.float8_e4m3
    xh = x.astype(f8t)
    xl = (x - xh.astype(np.float32)).astype(f8t)
    xblk = np.ascontiguousarray(
        np.concatenate(
            [xh.reshape(NBLK, P, 1, D), xl.reshape(NBLK, P, 1, D)], axis=2
        ).transpose(1, 0, 2, 3).reshape(P, NBLK * 2 * D)
    )

    in_maps = [
        {
            "x16s": xpad,
            "idx": prep["idx"][c],
            "inv": prep["inv"][c],
            "wstk": np.ascontiguousarray(
                np.vstack([weight, weight, bias[None], weight, bias[None]])
            ).astype(np.float32),
            "ident": np.eye(P, dtype=np.float32),
            "a8": prep["a8"][c].view(ml_dtypes.float8_e4m3),
            "xblk": xblk,
        }
        for c in range(NCORES)
    ]

    res = bass_utils.run_bass_kernel_spmd(
        nc, in_maps, core_ids=list(range(NCORES)), trace=_TRACE
    )
    LAST_RESULTS = res

    out = np.empty((N, D), dtype=np.float32)
    for c in range(NCORES):
        out[prep["perm"][c]] = res.results[c]["out"]
    return out
